# revision 3
# baseline (speedup 1.0000x reference)
"""Trainium2 Bass kernel for a channel-attention block.

Reference math (per batch sample, a: [C, N] with C=128 channels,
N = H*W spatial):
    b   = a @ a.T                  # [C, C] channel affinity (Gram)
    x   = softmax(b, axis=-1)
    c   = x @ a                    # [C, N]
    out = beta * c + a

Sharding: data-parallel over the batch dim — 16 samples / 8 cores =
2 samples per NeuronCore, no cross-core communication.

Single-HBM-pass design (per sample):
  stage A: gpsimd (SWDGE) DMA loads `a` in [128, LW] tiles, casting
           f32 -> bf16 in flight. The bf16 tiles stay RESIDENT in SBUF
           (16 MB/sample) so `a` is read from HBM exactly once. Each
           tile is PE-transposed in 128-col blocks into PSUM, copied
           back to SBUF (ACT/DVE), and Gram-accumulated into one PSUM
           bank via bf16 matmuls.
  stage B: row softmax on b (DVE max, ACT exp(+bias) with fused row
           sum, DVE reciprocal); the 1/rowsum normalization folds into
           the epilogue scalar bs = beta/rowsum. X is PE-transposed for
           use as the stage-C stationary operand.
  stage C: c = XT.T @ a_bf16 straight from the SBUF-resident tiles (no
           second HBM read); DVE epilogue out = (c * bs) + a_bf16 in
           bf16, stored to HBM as bf16 (host upcasts to f32).

HBM traffic per core: 64 MB read (f32 a, once) + 32 MB write (bf16
out) = 96 MB at ~358 GB/s => ~270 us floor. Stage C of sample s is
emission-interleaved with stage A of sample s+1; the shared cache pool
has n_loads + cache_extra slots so the next sample's loads can chase
stage C's consumption without stalling.
"""

import numpy as np

import concourse.bass as bass
import concourse.mybir as mybir
import concourse.tile as tile
from concourse import bacc
from concourse.bass_utils import run_bass_kernel_spmd
from concourse.masks import make_identity

F32 = mybir.dt.float32
BF16 = mybir.dt.bfloat16

N_CORES = 8
B, C, H, W = 16, 128, 256, 256
N_FULL = H * W
S = B // N_CORES  # samples per core


def build(S=S, C=C, N=N_FULL, LW=4096, TW=1024, MM_N=512, cache_extra=2,
          lead=2, out_dt="bf16", cps_dt="f32", eng_atcopy="alt",
          st_ring="sync", tp_bufs=3, cps_bufs=3, at_bufs=4):
    """Build + compile the per-core Bass program."""
    assert C == 128 and N % LW == 0 and LW % TW == 0 and TW % 128 == 0
    nc = bacc.Bacc("TRN2", target_bir_lowering=False, debug=False)

    a_d = nc.dram_tensor("a", [S, C, N], F32, kind="ExternalInput").ap()
    beta_d = nc.dram_tensor("beta", [C, 1], F32, kind="ExternalInput").ap()
    o_dt = BF16 if out_dt == "bf16" else F32
    c_dt = BF16 if cps_dt == "bf16" else F32
    out_d = nc.dram_tensor("out", [S, C, N], o_dt, kind="ExternalOutput").ap()

    n_loads = N // LW
    n_gram_mm = N // 128

    with tile.TileContext(nc) as tc:
        with (
            tc.tile_pool(name="const", bufs=1) as const_pool,
            tc.tile_pool(name="acache", bufs=n_loads + cache_extra) as cache_pool,
            tc.tile_pool(name="at", bufs=at_bufs) as at_pool,
            tc.tile_pool(name="sm", bufs=2) as sm_pool,
            tc.tile_pool(name="cout", bufs=3) as cout_pool,
            tc.tile_pool(name="tp_ps", bufs=tp_bufs, space="PSUM") as tp_psum,
            tc.tile_pool(name="gram_ps", bufs=2, space="PSUM") as gram_psum,
            tc.tile_pool(name="c_ps", bufs=cps_bufs, space="PSUM") as c_psum,
        ):
            ident_bf = const_pool.tile([128, 128], BF16, tag="identbf")
            make_identity(nc, ident_bf)
            beta_sb = const_pool.tile([C, 1], F32, tag="beta")
            nc.sync.dma_start(beta_sb, beta_d)

            def copy_op(engine_sel, idx, out, in_):
                """Route a copy/cast to ACT or DVE per engine_sel."""
                if engine_sel == "act" or (engine_sel == "alt" and idx % 2 == 0):
                    nc.scalar.copy(out, in_)
                else:
                    nc.vector.tensor_copy(out, in_)

            gram_state = {}   # s -> [b_ps, mm_count]
            xt_w = {}         # s -> lhsT weights for stage C
            beta_row = {}     # s -> beta/rowsum [C,1]
            cached = {}       # (s, j) -> SBUF-resident bf16 a tile

            def stage_a_chunk(s, j):
                """Cast-load tile j of sample s, transpose, Gram-accum."""
                if s not in gram_state:
                    b_ps = gram_psum.tile([C, C], F32, tag="gram",
                                          name=f"gram_{s}")
                    gram_state[s] = [b_ps, 0]
                st = gram_state[s]
                b_ps = st[0]
                abf = cache_pool.tile([C, LW], BF16, tag="acache",
                                      name=f"ac_{s}_{j}")
                cached[(s, j)] = abf
                # SWDGE cast-DMA: HBM f32 -> SBUF bf16 in one transfer.
                nc.gpsimd.dma_start(abf, a_d[s, :, j * LW:(j + 1) * LW])
                for g in range(LW // TW):
                    tp = tp_psum.tile([128, TW], BF16, tag="tp",
                                      name=f"tp_{s}_{j}_{g}")
                    for q in range(TW // 128):
                        nc.tensor.transpose(
                            tp[:, q * 128:(q + 1) * 128],
                            abf[:, g * TW + q * 128:g * TW + (q + 1) * 128],
                            ident_bf,
                        )
                    at_t = at_pool.tile([128, TW], BF16, tag="at",
                                        name=f"at_{s}_{j}_{g}")
                    copy_op(eng_atcopy, g, at_t, tp)
                    for q in range(TW // 128):
                        st[1] += 1
                        nc.tensor.matmul(
                            b_ps,
                            lhsT=at_t[:, q * 128:(q + 1) * 128],
                            rhs=at_t[:, q * 128:(q + 1) * 128],
                            start=(st[1] == 1),
                            stop=(st[1] == n_gram_mm),
                        )

            def softmax(s):
                """Unnormalized softmax: E = exp(b - rowmax), transposed for
                stage C. The 1/rowsum normalization folds into the epilogue
                scalar bs = beta/rowsum."""
                b_ps = gram_state[s][0]
                negm = sm_pool.tile([C, 1], F32, tag="negm", name=f"negm_{s}")
                nc.vector.tensor_reduce(
                    negm, b_ps, axis=mybir.AxisListType.X,
                    op=mybir.AluOpType.max, negate=True,
                )
                e_t = sm_pool.tile([C, C], BF16, tag="e", name=f"e_{s}")
                ssum = sm_pool.tile([C, 1], F32, tag="ssum", name=f"ssum_{s}")
                nc.scalar.activation(
                    e_t, b_ps, mybir.ActivationFunctionType.Exp,
                    bias=negm, accum_out=ssum,
                )
                xt_ps = tp_psum.tile([128, TW], BF16, tag="tp", name=f"xtp_{s}")
                nc.tensor.transpose(xt_ps[:, :128], e_t, ident_bf)
                xt_sb = sm_pool.tile([C, C], BF16, tag="xt", name=f"xt_{s}")
                nc.scalar.copy(xt_sb, xt_ps[:, :128])
                xt_w[s] = xt_sb
                rec = sm_pool.tile([C, 1], F32, tag="rec", name=f"rec_{s}")
                nc.vector.reciprocal(rec, ssum)
                bs = sm_pool.tile([C, 1], F32, tag="bs", name=f"bs_{s}")
                nc.vector.tensor_scalar_mul(bs, rec, beta_sb)
                beta_row[s] = bs

            def stage_c_chunk(s, j):
                """c = XT.T @ a_bf16 from SBUF; out = bs*c + a_bf16; store."""
                abf = cached.pop((s, j))
                obf = cout_pool.tile([C, LW], o_dt, tag="cout",
                                     name=f"cout_{s}_{j}")
                for q in range(LW // MM_N):
                    sl = slice(q * MM_N, (q + 1) * MM_N)
                    c_ps = c_psum.tile([128, MM_N], c_dt, tag="cps",
                                       name=f"cps_{s}_{j}_{q}")
                    nc.tensor.matmul(
                        c_ps, lhsT=xt_w[s], rhs=abf[:, sl],
                        start=True, stop=True,
                    )
                    nc.vector.scalar_tensor_tensor(
                        out=obf[:, sl],
                        in0=c_ps,
                        scalar=beta_row[s],
                        in1=abf[:, sl],
                        op0=mybir.AluOpType.mult,
                        op1=mybir.AluOpType.add,
                    )
                ring = nc.sync if st_ring == "sync" else nc.scalar
                ring.dma_start(out_d[s, :, j * LW:(j + 1) * LW], obf)

            # Software-pipelined emission: stage C of sample s runs `lead`
            # tiles ahead of stage A of sample s+1 on the PE queue, so
            # each cast-load (released by C(s) freeing its cache slot,
            # cache_extra tiles earlier) lands before PE needs it.
            for j in range(n_loads):
                stage_a_chunk(0, j)
            softmax(0)
            for s in range(1, S):
                ld = min(lead, n_loads)
                for j in range(ld):
                    stage_c_chunk(s - 1, j)
                for j in range(n_loads):
                    stage_a_chunk(s, j)
                    if j + ld < n_loads:
                        stage_c_chunk(s - 1, j + ld)
                softmax(s)
            for j in range(n_loads):
                stage_c_chunk(S - 1, j)

    nc.compile()
    return nc


_NC_CACHE: dict = {}


def _get_nc(**kw):
    key = tuple(sorted(kw.items()))
    if key not in _NC_CACHE:
        _NC_CACHE[key] = build(**kw)
    return _NC_CACHE[key]


def kernel(a, beta):
    """Full-input entry point: a [16,128,256,256] f32, beta [1] f32."""
    a = np.ascontiguousarray(np.asarray(a, dtype=np.float32))
    beta = np.asarray(beta, dtype=np.float32)
    nb, ch, h, w = a.shape
    n = h * w
    s = nb // N_CORES
    a3 = a.reshape(nb, ch, n)
    beta_b = np.broadcast_to(beta.reshape(1, 1), (ch, 1)).copy()

    nc = _get_nc(S=s, C=ch, N=n)
    in_maps = [
        {"a": a3[i * s:(i + 1) * s], "beta": beta_b} for i in range(N_CORES)
    ]
    res = run_bass_kernel_spmd(nc, in_maps, list(range(N_CORES)))
    out = np.concatenate(
        [np.asarray(res.results[i]["out"]) for i in range(N_CORES)], axis=0
    )
    return out.reshape(nb, ch, h, w).astype(np.float32)
